# revision 2
# baseline (speedup 1.0000x reference)
"""CRF negative-log-likelihood loss on 8 Trainium2 NeuronCores.

Shapes (hardcoded): inputs (2048, 512, 25) f32, tags (2048, 512) i64,
mask (2048, 512) i32 (all-ones in this problem config).

Algorithm: the log-partition (denominator) forward recurrence
    alpha_s = emit_s + logsumexp_i(alpha_{s-1} + trans)
is computed in the linear domain with a constant per-step rescale folded
into the transition matrix:  W_s = E_s * (kM^T W_{s-1}),
kM = exp(trans) * exp(-MU).  To halve the sequential depth, a forward
chain (from s=0) and a backward chain (from s=511) run simultaneously and
meet in the middle:
    log_den_b = log( W_255 . (kM V_256) ) + 511 * MU
Both chains for 2 batch-groups of 128 pack into one (100, 128) tile
(4 blocks x 25 tags), so each superstep is ONE 100x100x128 matmul (PE)
plus ONE elementwise multiply (DVE).  Data parallel over batch across the
8 cores (256 batch rows each).  The numerator (gold-path score) only
needs gathers and is computed on the host.
"""

import numpy as np

_B, _S, _T = 2048, 512, 25
_NCORES = 8
_BL = _B // _NCORES      # 256 batch rows per core
_NT = _S // 2            # 256 packed entries (init + 255 supersteps)
_P = 100                 # partitions: [fwd g0, fwd g1, bwd g0, bwd g1] x 25
_CB = 128                # batch columns per group
_MU = 3.7                # per-step rescale (log-domain)
_CHUNK = 32              # supersteps per DMA/exp chunk

_PROG = None


def _build_program():
    global _PROG
    if _PROG is not None:
        return _PROG
    from concourse import bacc, mybir, tile

    dt = mybir.dt
    AF = mybir.ActivationFunctionType
    nc = bacc.Bacc("TRN2", target_bir_lowering=False, debug=False)
    ep = nc.dram_tensor("ep", [_NT, _P, _CB], dt.float32, kind="ExternalInput")
    bd = nc.dram_tensor("bd", [_P, _P], dt.float32, kind="ExternalInput")
    sev = nc.dram_tensor("sev", [_P, 1], dt.float32, kind="ExternalInput")
    xout = nc.dram_tensor("xout", [_P, _CB], dt.float32, kind="ExternalOutput")
    with tile.TileContext(nc) as tc:
        with (
            tc.tile_pool(name="consts", bufs=1) as consts,
            tc.tile_pool(name="raw", bufs=2) as rawp,
            tc.tile_pool(name="emis", bufs=2) as epool,
            tc.tile_pool(name="x", bufs=2) as xpool,
            tc.tile_pool(name="q", bufs=2, space="PSUM") as qpool,
        ):
            bdt = consts.tile([_P, _P], dt.float32)
            nc.sync.dma_start(bdt[:], bd.ap())
            sevt = consts.tile([_P, 1], dt.float32)
            nc.sync.dma_start(sevt[:], sev.ap())
            x = None
            for c in range(_NT // _CHUNK):
                raw = rawp.tile([_P, _CHUNK, _CB], dt.float32)
                nc.sync.dma_start(
                    raw[:],
                    ep.ap()[c * _CHUNK:(c + 1) * _CHUNK].rearrange("k p c -> p k c"),
                )
                et = epool.tile([_P, _CHUNK, _CB], dt.float32)
                nc.scalar.activation(et[:], raw[:], AF.Exp)
                for k in range(_CHUNK):
                    esl = et[:, k, :]
                    if x is None:
                        x = xpool.tile([_P, _CB], dt.float32)
                        nc.vector.tensor_scalar_mul(x[:], esl, sevt[:])
                    else:
                        q = qpool.tile([_P, _CB], dt.float32)
                        nc.tensor.matmul(q[:], bdt[:], x[:], start=True, stop=True)
                        xn = xpool.tile([_P, _CB], dt.float32)
                        nc.vector.tensor_mul(xn[:], q[:], esl)
                        x = xn
            nc.sync.dma_start(xout.ap(), x[:])
    nc.compile()
    _PROG = nc
    return _PROG


def _pack_inputs(x):
    """(2048, 512, 25) f32 -> per-core (NT, 100, 128) packed E-logit entries."""
    z = np.ascontiguousarray(x.transpose(1, 0, 2))   # (512, 2048, 25)
    zf = z[:_NT]                                     # entry t -> s = t
    zb = z[:_S - _NT - 1:-1]                         # entry t -> s = 511 - t
    ep = np.empty((_NCORES, _NT, _P, _CB), np.float32)
    for n in range(_NCORES):
        for g in range(2):
            b0 = n * _BL + g * _CB
            ep[n, :, 25 * g:25 * g + 25, :] = zf[:, b0:b0 + _CB, :].transpose(0, 2, 1)
            ep[n, :, 50 + 25 * g:75 + 25 * g, :] = zb[:, b0:b0 + _CB, :].transpose(0, 2, 1)
    return ep


def _device_inputs(x, trans64, start64, end64):
    ep = _pack_inputs(x)
    kM32 = np.exp(trans64 - _MU).astype(np.float32)
    bd = np.zeros((_P, _P), np.float32)
    bd[0:25, 0:25] = kM32
    bd[25:50, 25:50] = kM32
    bd[50:75, 50:75] = kM32.T
    bd[75:100, 75:100] = kM32.T
    sev = np.empty((_P, 1), np.float32)
    sev[0:25, 0] = np.exp(start64)
    sev[25:50, 0] = np.exp(start64)
    sev[50:75, 0] = np.exp(end64)
    sev[75:100, 0] = np.exp(end64)
    return [{"ep": ep[n], "bd": bd, "sev": sev} for n in range(_NCORES)]


def _run_device(in_maps, **kw):
    from concourse.bass_utils import run_bass_kernel_spmd

    nc = _build_program()
    return run_bass_kernel_spmd(nc, in_maps, list(range(_NCORES)), **kw)


def _logden_sum(results, trans64):
    """Host-side meet of the two chains, in float64."""
    kM64 = np.exp(trans64 - _MU)
    total = 0.0
    for n in range(_NCORES):
        X = results[n]["xout"].astype(np.float64)    # (100, 128)
        W = X[0:50].reshape(2, 25, _CB)              # W_255 per group
        V = X[50:100].reshape(2, 25, _CB)            # V_256 per group
        kMV = np.einsum("jk,gkc->gjc", kM64, V)
        D = (W * kMV).sum(axis=1)                    # (2, 128)
        total += np.log(D).sum() + D.size * (_S - 1) * _MU
    return total


def _numerator(x, tags, trans64, start64, end64):
    num = start64[tags[:, 0]].sum() + end64[tags[:, -1]].sum()
    pair = (tags[:, :-1].astype(np.int64) * _T + tags[:, 1:]).ravel()
    cnt = np.bincount(pair, minlength=_T * _T)
    num += cnt @ trans64.ravel()
    emis = np.take_along_axis(x, tags[..., None].astype(np.int64), axis=2)
    return num + emis.sum(dtype=np.float64)


def _fallback(inputs, transitions, start_transitions, end_transitions, tags, mask):
    """Generic (masked) numpy reference path; used only for unexpected inputs."""
    logits = np.asarray(inputs, dtype=np.float64)
    maskf = np.asarray(mask, dtype=np.float64)
    tags = np.asarray(tags)
    trans = np.asarray(transitions, dtype=np.float64)
    start_t = np.asarray(start_transitions, dtype=np.float64)
    end_t = np.asarray(end_transitions, dtype=np.float64)
    B, S, T = logits.shape
    exp_trans = np.exp(trans)
    alpha = start_t[None, :] + logits[:, 0]
    for s in range(1, S):
        c = alpha.max(axis=1)
        w2 = np.exp(alpha - c[:, None]) @ exp_trans
        new_alpha = c[:, None] + np.log(w2) + logits[:, s]
        m = maskf[:, s][:, None]
        alpha = new_alpha * m + alpha * (1.0 - m)
    stops = alpha + end_t[None, :]
    smx = stops.max(axis=1)
    log_den = smx + np.log(np.exp(stops - smx[:, None]).sum(axis=1))
    score = start_t[tags[:, 0]]
    score = score + (trans[tags[:, :-1], tags[:, 1:]] * maskf[:, 1:]).sum(axis=1)
    emit_score = (
        np.take_along_axis(logits[:, :-1], tags[:, :-1, None], axis=2)[..., 0]
        * maskf[:, :-1]
    )
    score = score + emit_score.sum(axis=1)
    last_idx = maskf.sum(axis=1).astype(np.int64) - 1
    rows = np.arange(B)
    last_tags = tags[rows, last_idx]
    score = score + end_t[last_tags]
    score = score + logits[rows, S - 1, last_tags] * maskf[:, -1]
    return np.float32((score - log_den).sum())


def kernel(inputs, transitions, start_transitions, end_transitions, tags, mask):
    x = np.ascontiguousarray(np.asarray(inputs), dtype=np.float32)
    tags = np.asarray(tags)
    mask = np.asarray(mask)
    if x.shape != (_B, _S, _T) or not (mask == 1).all():
        return _fallback(inputs, transitions, start_transitions,
                         end_transitions, tags, mask)
    trans64 = np.asarray(transitions, dtype=np.float64)
    start64 = np.asarray(start_transitions, dtype=np.float64)
    end64 = np.asarray(end_transitions, dtype=np.float64)

    in_maps = _device_inputs(x, trans64, start64, end64)
    res = _run_device(in_maps)
    log_den = _logden_sum(res.results, trans64)
    num = _numerator(x, tags, trans64, start64, end64)
    return np.float32(num - log_den)
